# revision 53
# baseline (speedup 1.0000x reference)
"""Trainium2 Bass kernel for CFConv-style GNN message passing layer.

Pipeline (per core, fully independent - no collectives):
  edge phase: edges of this core's dst slice, ordered (src-bucket,
           dst-block) with run lengths padded to a cross-core common
           profile. Per 4096-slot chunk: transpose-mode ucode dma_gather
           of x rows in bf16 (256B each, int16 idx within bucket, output
           feature-major [128 dims, slots]), per-tile h = x_gath.T @
           lin1_w.T on PE (no separate h phase at all - gathers start at
           t=0), bf16 edge-MLP for W (feature-major, A/B stacked,
           cosine-cutoff folded into the exp via a lnC poly row) + PE
           transpose to edge-major, msg = h_gath*W, then selection-matmul
           reduce: sel[k, m] = (segid[k] == m) built on DVE in bf16,
           sel.T @ msg accumulated in PSUM per (bucket, dst-block) run,
           added into SBUF agg.
  y phase: out = x + relu(agg @ lin2_w.T + b) via PE transpose + matmul.
"""

import math

import numpy as np

DIM = 128
NF = 64
NG = 50
CUTOFF = 5.0
N_CORES = 8
N_BUCKETS = 4
CHUNK = 4096          # gather chunk (slots per dma_gather call)
AHEAD = 4             # gather chunks in flight


def _cfg(n_nodes, n_cores):
    npc = n_nodes // n_cores
    assert npc * n_cores == n_nodes
    npad = ((npc + 127) // 128) * 128
    nblk = npad // 128
    nbg = ((n_nodes + 2047) // 2048) * 2048   # global padded (h phase, /512)
    bkt = nbg // N_BUCKETS
    assert bkt <= 32767
    return dict(npc=npc, npad=npad, nblk=nblk, n_nodes=n_nodes,
                nbg=nbg, bkt=bkt)


def prep_host(x, edge_index, edge_weight, edge_attr, n_cores=N_CORES):
    """Shard edges by dst core; order (bucket, dst_block, dst); build
    common run-length profile and per-core input arrays."""
    import ml_dtypes
    bf = ml_dtypes.bfloat16
    cfg = _cfg(x.shape[0], n_cores)
    npc, npad, nblk = cfg["npc"], cfg["npad"], cfg["nblk"]

    src = np.asarray(edge_index[0], dtype=np.int64)
    dst = np.asarray(edge_index[1], dtype=np.int64)
    d = np.asarray(edge_attr, dtype=np.float32)

    bkt = cfg["bkt"]
    core_of = dst // npc
    per_core = []
    runlens = np.zeros((n_cores, N_BUCKETS, nblk), dtype=np.int64)
    for r in range(n_cores):
        m = core_of == r
        s, dl, dd = src[m], dst[m] - r * npc, d[m]
        bu = s // bkt
        db = dl // 128
        order = np.lexsort((dl, db, bu))
        s, dl, dd, bu, db = s[order], dl[order], dd[order], bu[order], db[order]
        cnt = np.zeros((N_BUCKETS, nblk), dtype=np.int64)
        np.add.at(cnt, (bu, db), 1)
        runlens[r] = cnt
        per_core.append((s, dl, dd, bu, db, cnt))

    runlen = runlens.max(axis=0)                      # common profile
    bucket_tot = runlen.sum(axis=1)
    bucket_pad = ((bucket_tot + CHUNK - 1) // CHUNK) * CHUNK
    run_off = np.zeros((N_BUCKETS, nblk), dtype=np.int64)
    off = 0
    for b in range(N_BUCKETS):
        start = off
        for db in range(nblk):
            run_off[b, db] = off
            off += runlen[b, db]
        off = start + bucket_pad[b]
    e_pad = int(off)
    assert e_pad % CHUNK == 0
    cfg["e_pad"] = e_pad
    cfg["runlen"] = runlen
    cfg["run_off"] = run_off
    cfg["bucket_pad"] = bucket_pad

    ins = []
    for r in range(n_cores):
        s, dl, dd, bu, db, cnt = per_core[r]
        srcl = np.zeros(e_pad, dtype=np.int16)
        dpad = np.full(e_pad, CUTOFF, dtype=np.float32)
        segid = np.full(e_pad, 272.0, dtype=np.float32)
        pos = 0
        run_i = 0
        for b in range(N_BUCKETS):
            for k in range(nblk):
                n = int(cnt[b, k])
                o = int(run_off[b, k])
                sl = slice(pos, pos + n)
                srcl[o:o + n] = (s[sl] - b * bkt).astype(np.int16)
                dpad[o:o + n] = dd[sl]
                segid[o:o + n] = (dl[sl] - k * 128 +
                                  128 * (run_i % 2)).astype(np.float32)
                pos += n
                run_i += 1
        cw = (0.5 * (np.cos(dpad * (math.pi / CUTOFF)) + 1.0)).astype(np.float32)
        lnc = np.log(np.maximum(cw, 1e-30)).astype(np.float32)
        np.maximum(lnc, -80.0, out=lnc)
        nt = e_pad // 2048
        poly = np.empty((nt, 6, 1024), dtype=np.float32)
        d2 = dpad * dpad
        dv = dpad.reshape(nt, 2, 1024)
        d2v = d2.reshape(nt, 2, 1024).astype(np.float32)
        lncv = lnc.reshape(nt, 2, 1024)
        poly[:, 0, :] = dv[:, 0, :]
        poly[:, 1, :] = d2v[:, 0, :]
        poly[:, 2, :] = lncv[:, 0, :]
        poly[:, 3, :] = dv[:, 1, :]
        poly[:, 4, :] = d2v[:, 1, :]
        poly[:, 5, :] = lncv[:, 1, :]
        xt_own = np.zeros((DIM, npad), dtype=np.float32)
        xt_own[:, :npc] = np.asarray(x[r * npc:(r + 1) * npc, :], np.float32).T
        ins.append(dict(
            xT=np.ascontiguousarray(xt_own),
            poly=poly,
            segid=np.ascontiguousarray(
                segid.reshape(-1, 128).T).astype(bf),      # [128, e/128]
            srcidx=np.ascontiguousarray(np.tile(srcl.reshape(-1, 16).T, (8, 1))),
        ))
    return cfg, ins


def prep_weights(cfg, x, lin1_w, lin2_w, lin2_b, enn1_w, enn1_b, enn2_w, enn2_b):
    import ml_dtypes
    bf = ml_dtypes.bfloat16
    offset = np.linspace(0.0, CUTOFF, NG).astype(np.float32)
    coeff = np.float32(-0.5 / (offset[1] - offset[0]) ** 2)
    poly_lhsT = np.zeros((6, 128), dtype=np.float32)
    for r0, c0 in ((0, 0), (3, 64)):
        poly_lhsT[r0 + 0, c0:c0 + NG] = -2.0 * coeff * offset
        poly_lhsT[r0 + 1, c0:c0 + NG] = coeff
        poly_lhsT[r0 + 2, c0:c0 + NG] = 1.0
    eb = np.full((128, 1), -88.0, dtype=np.float32)
    eb[:NG, 0] = coeff * offset * offset
    eb[64:64 + NG, 0] = coeff * offset * offset
    b1s = np.zeros((128, 1), dtype=np.float32)
    b1s[:NF, 0] = enn1_b
    b1s[64:64 + NF, 0] = enn1_b
    b2s = np.zeros((128, 1), dtype=np.float32)
    b2s[:NF, 0] = enn2_b
    b2s[64:64 + NF, 0] = enn2_b

    def _stack2(w, rows):
        out = np.zeros((rows, w.shape[1]), dtype=np.float32)
        out[:w.shape[0]] = w
        out[64:64 + w.shape[0]] = w
        return out

    xb = np.zeros((cfg["nbg"], DIM), dtype=np.float32)
    xb[:cfg["n_nodes"], :] = np.asarray(x, np.float32)
    iota_f = np.broadcast_to(np.arange(128, dtype=np.float32)[None, :],
                             (128, 128)).copy()
    iota_f2 = iota_f + 128.0
    return dict(
        xb=xb.astype(bf),
        lin1_wT=np.ascontiguousarray(lin1_w.T.astype(np.float32)).astype(bf),
        lin2_wT=np.ascontiguousarray(lin2_w.T.astype(np.float32)).astype(bf),
        enn1_wT=_stack2(enn1_w.T.astype(np.float32), 114).astype(bf),
        enn2_wT=_stack2(enn2_w.T.astype(np.float32), 128).astype(bf),
        poly_lhsT=poly_lhsT,
        eb=eb, b1s=b1s, b2s=b2s,
        identb=np.eye(128, dtype=np.float32).astype(bf),
        ident=np.eye(128, dtype=np.float32),
        iota_f=iota_f.astype(bf), iota_f2=iota_f2.astype(bf),
        l2b=np.ascontiguousarray(lin2_b.astype(np.float32).reshape(128, 1)),
    )


def build_nc(cfg, n_cores=N_CORES):
    import concourse.bass as bass
    import concourse.bacc as bacc
    import concourse.mybir as mybir
    import concourse.tile as tile
    from concourse import library_config

    f32 = mybir.dt.float32
    bf16 = mybir.dt.bfloat16
    i16 = mybir.dt.int16
    npad, nblk, e_pad = cfg["npad"], cfg["nblk"], cfg["e_pad"]
    runlen, run_off = cfg["runlen"], cfg["run_off"]
    NBG, BKT = cfg["nbg"], cfg["bkt"]
    NT = e_pad // 2048
    NCHUNK = e_pad // CHUNK

    nc = bacc.Bacc(None, num_devices=n_cores)

    xT_d = nc.dram_tensor("xT", [DIM, npad], f32, kind="ExternalInput")
    xb_d = nc.dram_tensor("xb", [NBG, DIM], bf16, kind="ExternalInput")
    poly_d = nc.dram_tensor("poly", [NT, 6, 1024], f32, kind="ExternalInput")
    seg_d = nc.dram_tensor("segid", [128, e_pad // 128], bf16,
                           kind="ExternalInput")
    sidx_d = nc.dram_tensor("srcidx", [128, e_pad // 16], i16,
                            kind="ExternalInput")
    w_d = {}
    for name, shape, dt in [
            ("lin1_wT", [DIM, NF], bf16), ("lin2_wT", [NF, DIM], bf16),
            ("enn1_wT", [114, NF], bf16), ("enn2_wT", [128, NF], bf16),
            ("poly_lhsT", [6, 128], f32), ("eb", [128, 1], f32),
            ("b1s", [128, 1], f32), ("b2s", [128, 1], f32),
            ("l2b", [128, 1], f32), ("identb", [128, 128], bf16),
            ("ident", [128, 128], f32), ("iota_f", [128, 128], bf16),
            ("iota_f2", [128, 128], bf16)]:
        w_d[name] = nc.dram_tensor(name, shape, dt, kind="ExternalInput")
    out_d = nc.dram_tensor("out", [DIM, npad], f32, kind="ExternalOutput")

    # bucket of each chunk + real (non-pad) slots per chunk (compile-time)
    chunk_bucket = []
    nreal = []
    for bb in range(N_BUCKETS):
        tot = int(runlen[bb].sum())
        sz = int(cfg["bucket_pad"][bb])
        for gg in range(sz // CHUNK):
            chunk_bucket.append(bb)
            nreal.append(min(CHUNK, max(0, tot - gg * CHUNK)))
    assert len(nreal) == NCHUNK

    # compile-time run table: for each chunk, list of
    # (tile, dblock, first, last, parity) spans.
    spans_by_chunk = [[] for _ in range(NCHUNK)]
    last_chunk_of_blk = [0] * nblk
    run_i = 0
    for b in range(N_BUCKETS):
        for k in range(nblk):
            L = int(runlen[b, k])
            if L == 0:
                run_i += 1
                continue
            o = int(run_off[b, k])
            first = True
            pos = o
            while pos < o + L:
                t128 = pos // 128
                k0 = pos % 128
                k1 = min(128, k0 + (o + L - pos))
                g = (t128 * 128) // CHUNK
                pos += k1 - k0
                last = pos >= o + L
                spans_by_chunk[g].append(
                    (t128 - g * 32, k, first, last, run_i % 2))
                first = False
            last_chunk_of_blk[k] = max(last_chunk_of_blk[k],
                                       ((o + L - 1) // 128 * 128) // CHUNK)
            run_i += 1

    # y step s (dst blocks 2s, 2s+1) can run once this chunk is computed
    ysteps_by_chunk = {}
    for s in range(npad // 256):
        blks = [min(2 * s, nblk - 1), min(2 * s + 1, nblk - 1)]
        g = max(last_chunk_of_blk[b_] for b_ in blks)
        ysteps_by_chunk.setdefault(g, []).append(s)

    with tile.TileContext(nc) as tc:
        with (tc.tile_pool(name="const", bufs=1) as cp,
              tc.tile_pool(name="pg", bufs=AHEAD + 1) as pg,
              tc.tile_pool(name="ep", bufs=2) as ep,
              tc.tile_pool(name="yp", bufs=3) as yp,
              tc.tile_pool(name="pp_a", bufs=1, space="PSUM") as pp_a,
              tc.tile_pool(name="pp_b", bufs=1, space="PSUM") as pp_b,
              tc.tile_pool(name="pp_c", bufs=1, space="PSUM") as pp_c,
              tc.tile_pool(name="pp_g", bufs=1, space="PSUM") as pp_g,
              tc.tile_pool(name="pp_s", bufs=2, space="PSUM") as pp_s):
            wt = {}
            for name in w_d:
                t = cp.tile(list(w_d[name].shape), w_d[name].dtype,
                            tag='w_' + name)
                nc.sync.dma_start(out=t[:], in_=w_d[name][:, :])
                wt[name] = t
            nc.gpsimd.load_library(library_config.mlp)
            # agg split into 8 block-range tiles so late y-step READS of
            # low blocks don't create whole-tile WARs against the remaining
            # flush-add WRITES of high blocks (Tile tracks deps per tile).
            AGG_G = 13
            n_aggt = (nblk + AGG_G - 1) // AGG_G
            agg_t = []
            for q in range(n_aggt):
                sz = min(AGG_G, nblk - q * AGG_G)
                t = cp.tile([128, sz, NF], f32, tag=f'agg{q}')
                nc.vector.memset(t[:], 0.0)
                agg_t.append(t)

            def agg_of(db):
                return agg_t[db // AGG_G], db % AGG_G

            # ---------------- edge phase ----------------
            pend = {}

            def load_chunk(g):
                b = chunk_bucket[g]
                sidx = pg.tile([128, 256], i16, tag='sidx')
                nc.sync.dma_start(
                    out=sidx[:], in_=sidx_d[:, g * 256:(g + 1) * 256])
                segt = pg.tile([128, 32], bf16, tag='segt')
                nc.sync.dma_start(
                    out=segt[:], in_=seg_d[:, g * 32:(g + 1) * 32])
                # transpose-mode gather of x rows: gx[d, slot] = x[src, d]
                gx = pg.tile([128, 1, CHUNK], bf16, tag='gx')
                nidx = ((nreal[g] + 127) // 128) * 128
                if nidx < CHUNK:
                    nc.vector.memset(gx[:, :, nidx:], 0.0)
                if nidx > 0:
                    nc.gpsimd.dma_gather(
                        gx[:, :, :nidx],
                        xb_d[b * BKT:(b + 1) * BKT, :],
                        sidx[:, :nidx // 16],
                        num_idxs=nidx, num_idxs_reg=nidx, elem_size=DIM,
                        transpose=True, single_packet=False)
                pend[g] = (gx, segt)

            agg_ps = {}   # dblock -> psum tile (open accumulations)

            def compute_chunk(g):
                gx, segt = pend.pop(g)
                msg = ep.tile([128, 32, NF], bf16, tag='msg')
                for half in range(2):
                    if half * 2048 >= nreal[g]:
                        continue
                    t = 2 * g + half
                    poly = ep.tile([6, 1024], f32, tag='poly')
                    nc.sync.dma_start(out=poly[:], in_=poly_d[t, :, :])
                    ppsum = pp_a.tile([128, 1024], f32, tag='ppsum')
                    for n5 in range(2):
                        nc.tensor.matmul(
                            ppsum[:, n5 * 512:(n5 + 1) * 512],
                            lhsT=wt["poly_lhsT"][:],
                            rhs=poly[:, n5 * 512:(n5 + 1) * 512],
                            start=True, stop=True)
                    smear = ep.tile([128, 1024], bf16, tag='smear')
                    nc.scalar.activation(
                        smear[:], ppsum[:],
                        mybir.ActivationFunctionType.Exp, bias=wt["eb"][:])
                    h1p = pp_b.tile([128, 1024], f32, tag='h1p')
                    for sub in range(2):
                        for n5 in range(2):
                            nc.tensor.matmul(
                                h1p[sub * 64:(sub + 1) * 64,
                                    n5 * 512:(n5 + 1) * 512],
                                lhsT=wt["enn1_wT"][sub * 64:sub * 64 + NG, :],
                                rhs=smear[sub * 64:sub * 64 + NG,
                                          n5 * 512:(n5 + 1) * 512],
                                start=True, stop=True)
                    h1r = ep.tile([128, 1024], bf16, tag='h1r')
                    nc.scalar.activation(
                        h1r[:], h1p[:], mybir.ActivationFunctionType.Relu,
                        bias=wt["b1s"][:])
                    wtp = pp_a.tile([128, 1024], f32, tag='ppsum', name='wtp')
                    for sub in range(2):
                        for n5 in range(2):
                            nc.tensor.matmul(
                                wtp[sub * 64:(sub + 1) * 64,
                                    n5 * 512:(n5 + 1) * 512],
                                lhsT=wt["enn2_wT"][sub * 64:(sub + 1) * 64, :],
                                rhs=h1r[sub * 64:(sub + 1) * 64,
                                        n5 * 512:(n5 + 1) * 512],
                                start=True, stop=True)
                    wts = ep.tile([128, 1024], bf16, tag='wts')
                    nc.scalar.activation(
                        wts[:], wtp[:],
                        mybir.ActivationFunctionType.Identity,
                        bias=wt["b2s"][:])
                    wcp = pp_c.tile([128, 1024], bf16, tag='wcp')
                    for c in range(8):
                        nc.tensor.transpose(
                            wcp[:, c * 128:(c + 1) * 128],
                            wts[:, c * 128:(c + 1) * 128], wt["identb"][:])
                    wcv = wcp[:].rearrange("p (c k) -> p c k", k=128)
                    for sub in range(2):
                        j0 = half * 16 + sub * 8
                        # h rows for these 8 tiles: h[slot, f] = sum_d
                        # gx[d, slot] * lin1_wT[d, f], per 128-slot tile
                        hg = pp_g.tile([128, 512], f32, tag='hg',
                                       name=f'hg_{g}_{half}_{sub}')
                        for t8 in range(8):
                            t = j0 + t8
                            nc.tensor.matmul(
                                hg[:, t8 * 64:(t8 + 1) * 64],
                                lhsT=gx[:, 0, t * 128:(t + 1) * 128],
                                rhs=wt["lin1_wT"][:],
                                start=True, stop=True)
                        hgs = ep.tile([128, 8, NF], f32, tag='hgs')
                        nc.scalar.activation(
                            hgs[:], hg[:].rearrange("p (c f) -> p c f", f=NF),
                            mybir.ActivationFunctionType.Copy)
                        mslice = msg[:, j0:j0 + 8, :]
                        nc.vector.tensor_tensor(
                            out=mslice,
                            in0=hgs[:],
                            in1=wcv[:, :, sub * 64:(sub + 1) * 64],
                            op=mybir.AluOpType.mult)
                # sel generation (both parities): [128, 32, 128] bf16
                sel0 = ep.tile([128, 32, 128], bf16, tag='sel0')
                nc.vector.tensor_tensor(
                    out=sel0[:],
                    in0=wt["iota_f"][:].unsqueeze(1)
                        .to_broadcast([128, 32, 128]),
                    in1=segt[:].unsqueeze(2)
                        .to_broadcast([128, 32, 128]),
                    op=mybir.AluOpType.is_equal)
                sel1 = ep.tile([128, 32, 128], bf16, tag='sel1')
                nc.vector.tensor_tensor(
                    out=sel1[:],
                    in0=wt["iota_f2"][:].unsqueeze(1)
                        .to_broadcast([128, 32, 128]),
                    in1=segt[:].unsqueeze(2)
                        .to_broadcast([128, 32, 128]),
                    op=mybir.AluOpType.is_equal)
                # sel-reduce with immediate flush at each run's end
                for (t, db, first, last, par) in spans_by_chunk[g]:
                    if first:
                        psnew = pp_s.tile([128, 256], f32, tag='aggps',
                                          name=f'aggps_{g}_{t}_{db}')[:, :NF]
                        agg_ps[db] = psnew
                    ps = agg_ps[db]
                    sel = sel1 if par else sel0
                    nc.tensor.matmul(
                        ps[:],
                        lhsT=sel[:, t, :],
                        rhs=msg[:, t, :],
                        start=first, stop=last)
                    if last:
                        agg_ps.pop(db)
                        at, loc = agg_of(db)
                        nc.vector.tensor_tensor(
                            out=at[:, loc, :], in0=at[:, loc, :],
                            in1=ps[:], op=mybir.AluOpType.add)

            # y PSUM comes exclusively from pp_b (every WAR points backward
            # in program order - never pp_s, whose open aggps accumulators
            # would deadlock the in-order PE queue).
            def y_step(s):
                atp = pp_b.tile([128, 1024], f32, tag='h1p',
                                name=f'atp_{s}')[:64, :256]
                for j in range(2):
                    blk = 2 * s + j
                    at, loc = agg_of(blk)
                    nc.tensor.transpose(
                        atp[:, j * 128:(j + 1) * 128],
                        at[:, loc, :], wt["ident"][:])
                ats = yp.tile([64, 256], bf16, tag='ats')
                nc.scalar.activation(ats[:], atp[:],
                                     mybir.ActivationFunctionType.Copy)
                ytp = pp_b.tile([128, 1024], f32, tag='h1p',
                                name=f'ytp_{s}')[:, :256]
                nc.tensor.matmul(ytp[:], lhsT=wt["lin2_wT"][:],
                                 rhs=ats[:], start=True, stop=True)
                yr = yp.tile([128, 256], f32, tag='yr')
                nc.scalar.activation(yr[:], ytp[:],
                                     mybir.ActivationFunctionType.Relu,
                                     bias=wt["l2b"][:])
                xt2 = yp.tile([128, 256], f32, tag='xt2')
                nc.sync.dma_start(out=xt2[:],
                                  in_=xT_d[:, s * 256:(s + 1) * 256])
                ot = yp.tile([128, 256], f32, tag='ot')
                nc.vector.tensor_tensor(out=ot[:], in0=yr[:], in1=xt2[:],
                                        op=mybir.AluOpType.add)
                nc.sync.dma_start(out=out_d[:, s * 256:(s + 1) * 256],
                                  in_=ot[:])

            # y steps burst only once ALL gathers are issued (iteration
            # >= NCHUNK - AHEAD): injected engine work can no longer stall
            # the gather stream, and the burst hides under the final
            # gathers' execution window.
            ready_s = 0
            for g in range(min(AHEAD, NCHUNK)):
                load_chunk(g)
            for g in range(NCHUNK):
                if g + AHEAD < NCHUNK:
                    load_chunk(g + AHEAD)
                compute_chunk(g)
                if g >= NCHUNK - AHEAD:
                    while (ready_s < npad // 256 and
                           max(last_chunk_of_blk[min(2 * ready_s, nblk - 1)],
                               last_chunk_of_blk[
                                   min(2 * ready_s + 1, nblk - 1)]) <= g):
                        y_step(ready_s)
                        ready_s += 1

            for s in range(ready_s, npad // 256):
                y_step(s)
    nc.compile()
    return nc


def run(inputs, n_cores=N_CORES, trace=False, **_ignored):
    from concourse.bass_utils import run_bass_kernel_spmd

    x = np.asarray(inputs["x"], np.float32)
    cfg, per_core = prep_host(x, inputs["edge_index"], inputs["edge_weight"],
                              inputs["edge_attr"], n_cores)
    wts = prep_weights(cfg, x, inputs["lin1_w"], inputs["lin2_w"],
                       inputs["lin2_b"], inputs["enn1_w"], inputs["enn1_b"],
                       inputs["enn2_w"], inputs["enn2_b"])
    nc = build_nc(cfg, n_cores)
    in_maps = [dict(per_core[r], **wts) for r in range(n_cores)]
    res = run_bass_kernel_spmd(nc, in_maps, core_ids=list(range(n_cores)),
                               trace=trace)
    npc = cfg["npc"]
    out = np.concatenate(
        [np.asarray(res.results[r]["out"]).reshape(DIM, cfg["npad"])[:, :npc].T
         for r in range(n_cores)], axis=0)
    return out, res


def kernel(**inputs):
    out, _ = run(inputs)
    return out


# revision 54
# speedup vs baseline: 1.0041x; 1.0041x over previous
"""Trainium2 Bass kernel for CFConv-style GNN message passing layer.

Pipeline (per core, fully independent - no collectives):
  edge phase: edges of this core's dst slice, ordered (src-bucket,
           dst-block) with run lengths padded to a cross-core common
           profile. Per 4096-slot chunk: transpose-mode ucode dma_gather
           of x rows in bf16 (256B each, int16 idx within bucket, output
           feature-major [128 dims, slots]), per-tile h = x_gath.T @
           lin1_w.T on PE (no separate h phase at all - gathers start at
           t=0), bf16 edge-MLP for W (feature-major, A/B stacked,
           cosine-cutoff folded into the exp via a lnC poly row) + PE
           transpose to edge-major, msg = h_gath*W, then selection-matmul
           reduce: sel[k, m] = (segid[k] == m) built on DVE in bf16,
           sel.T @ msg accumulated in PSUM per (bucket, dst-block) run,
           added into SBUF agg.
  y phase: out = x + relu(agg @ lin2_w.T + b) via PE transpose + matmul.
"""

import math

import numpy as np

DIM = 128
NF = 64
NG = 50
CUTOFF = 5.0
N_CORES = 8
N_BUCKETS = 4
CHUNK = 4096          # gather chunk (slots per dma_gather call)
AHEAD = 4             # gather chunks in flight


def _cfg(n_nodes, n_cores):
    npc = n_nodes // n_cores
    assert npc * n_cores == n_nodes
    npad = ((npc + 127) // 128) * 128
    nblk = npad // 128
    nbg = ((n_nodes + 2047) // 2048) * 2048   # global padded (h phase, /512)
    bkt = nbg // N_BUCKETS
    assert bkt <= 32767
    return dict(npc=npc, npad=npad, nblk=nblk, n_nodes=n_nodes,
                nbg=nbg, bkt=bkt)


def prep_host(x, edge_index, edge_weight, edge_attr, n_cores=N_CORES):
    """Shard edges by dst core; order (bucket, dst_block, dst); build
    common run-length profile and per-core input arrays."""
    import ml_dtypes
    bf = ml_dtypes.bfloat16
    cfg = _cfg(x.shape[0], n_cores)
    npc, npad, nblk = cfg["npc"], cfg["npad"], cfg["nblk"]

    src = np.asarray(edge_index[0], dtype=np.int64)
    dst = np.asarray(edge_index[1], dtype=np.int64)
    d = np.asarray(edge_attr, dtype=np.float32)

    bkt = cfg["bkt"]
    core_of = dst // npc
    per_core = []
    runlens = np.zeros((n_cores, N_BUCKETS, nblk), dtype=np.int64)
    for r in range(n_cores):
        m = core_of == r
        s, dl, dd = src[m], dst[m] - r * npc, d[m]
        bu = s // bkt
        db = dl // 128
        order = np.lexsort((dl, db, bu))
        s, dl, dd, bu, db = s[order], dl[order], dd[order], bu[order], db[order]
        cnt = np.zeros((N_BUCKETS, nblk), dtype=np.int64)
        np.add.at(cnt, (bu, db), 1)
        runlens[r] = cnt
        per_core.append((s, dl, dd, bu, db, cnt))

    runlen = runlens.max(axis=0)                      # common profile
    bucket_tot = runlen.sum(axis=1)
    bucket_pad = ((bucket_tot + CHUNK - 1) // CHUNK) * CHUNK
    run_off = np.zeros((N_BUCKETS, nblk), dtype=np.int64)
    off = 0
    for b in range(N_BUCKETS):
        start = off
        for db in range(nblk):
            run_off[b, db] = off
            off += runlen[b, db]
        off = start + bucket_pad[b]
    e_pad = int(off)
    assert e_pad % CHUNK == 0
    cfg["e_pad"] = e_pad
    cfg["runlen"] = runlen
    cfg["run_off"] = run_off
    cfg["bucket_pad"] = bucket_pad

    ins = []
    for r in range(n_cores):
        s, dl, dd, bu, db, cnt = per_core[r]
        srcl = np.zeros(e_pad, dtype=np.int16)
        dpad = np.full(e_pad, CUTOFF, dtype=np.float32)
        segid = np.full(e_pad, 272.0, dtype=np.float32)
        pos = 0
        run_i = 0
        for b in range(N_BUCKETS):
            for k in range(nblk):
                n = int(cnt[b, k])
                o = int(run_off[b, k])
                sl = slice(pos, pos + n)
                srcl[o:o + n] = (s[sl] - b * bkt).astype(np.int16)
                dpad[o:o + n] = dd[sl]
                segid[o:o + n] = (dl[sl] - k * 128 +
                                  128 * (run_i % 2)).astype(np.float32)
                pos += n
                run_i += 1
        cw = (0.5 * (np.cos(dpad * (math.pi / CUTOFF)) + 1.0)).astype(np.float32)
        lnc = np.log(np.maximum(cw, 1e-30)).astype(np.float32)
        np.maximum(lnc, -80.0, out=lnc)
        nt = e_pad // 2048
        poly = np.empty((nt, 6, 1024), dtype=np.float32)
        d2 = dpad * dpad
        dv = dpad.reshape(nt, 2, 1024)
        d2v = d2.reshape(nt, 2, 1024).astype(np.float32)
        lncv = lnc.reshape(nt, 2, 1024)
        poly[:, 0, :] = dv[:, 0, :]
        poly[:, 1, :] = d2v[:, 0, :]
        poly[:, 2, :] = lncv[:, 0, :]
        poly[:, 3, :] = dv[:, 1, :]
        poly[:, 4, :] = d2v[:, 1, :]
        poly[:, 5, :] = lncv[:, 1, :]
        xt_own = np.zeros((DIM, npad), dtype=np.float32)
        xt_own[:, :npc] = np.asarray(x[r * npc:(r + 1) * npc, :], np.float32).T
        ins.append(dict(
            xT=np.ascontiguousarray(xt_own),
            poly=poly,
            segid=np.ascontiguousarray(
                segid.reshape(-1, 128).T).astype(bf),      # [128, e/128]
            srcidx=np.ascontiguousarray(np.tile(srcl.reshape(-1, 16).T, (8, 1))),
        ))
    return cfg, ins


def prep_weights(cfg, x, lin1_w, lin2_w, lin2_b, enn1_w, enn1_b, enn2_w, enn2_b):
    import ml_dtypes
    bf = ml_dtypes.bfloat16
    offset = np.linspace(0.0, CUTOFF, NG).astype(np.float32)
    coeff = np.float32(-0.5 / (offset[1] - offset[0]) ** 2)
    poly_lhsT = np.zeros((6, 128), dtype=np.float32)
    for r0, c0 in ((0, 0), (3, 64)):
        poly_lhsT[r0 + 0, c0:c0 + NG] = -2.0 * coeff * offset
        poly_lhsT[r0 + 1, c0:c0 + NG] = coeff
        poly_lhsT[r0 + 2, c0:c0 + NG] = 1.0
    eb = np.full((128, 1), -88.0, dtype=np.float32)
    eb[:NG, 0] = coeff * offset * offset
    eb[64:64 + NG, 0] = coeff * offset * offset
    b1s = np.zeros((128, 1), dtype=np.float32)
    b1s[:NF, 0] = enn1_b
    b1s[64:64 + NF, 0] = enn1_b
    b2s = np.zeros((128, 1), dtype=np.float32)
    b2s[:NF, 0] = enn2_b
    b2s[64:64 + NF, 0] = enn2_b

    def _stack2(w, rows):
        out = np.zeros((rows, w.shape[1]), dtype=np.float32)
        out[:w.shape[0]] = w
        out[64:64 + w.shape[0]] = w
        return out

    xb = np.zeros((cfg["nbg"], DIM), dtype=np.float32)
    xb[:cfg["n_nodes"], :] = np.asarray(x, np.float32)
    iota_f = np.broadcast_to(np.arange(128, dtype=np.float32)[None, :],
                             (128, 128)).copy()
    iota_f2 = iota_f + 128.0
    return dict(
        xb=xb.astype(bf),
        lin1_wT=np.ascontiguousarray(lin1_w.T.astype(np.float32)).astype(bf),
        lin2_wT=np.ascontiguousarray(lin2_w.T.astype(np.float32)).astype(bf),
        enn1_wT=_stack2(enn1_w.T.astype(np.float32), 114).astype(bf),
        enn2_wT=_stack2(enn2_w.T.astype(np.float32), 128).astype(bf),
        poly_lhsT=poly_lhsT,
        eb=eb, b1s=b1s, b2s=b2s,
        identb=np.eye(128, dtype=np.float32).astype(bf),
        ident=np.eye(128, dtype=np.float32),
        iota_f=iota_f.astype(bf), iota_f2=iota_f2.astype(bf),
        l2b=np.ascontiguousarray(lin2_b.astype(np.float32).reshape(128, 1)),
    )


def build_nc(cfg, n_cores=N_CORES):
    import concourse.bass as bass
    import concourse.bacc as bacc
    import concourse.mybir as mybir
    import concourse.tile as tile
    from concourse import library_config

    f32 = mybir.dt.float32
    bf16 = mybir.dt.bfloat16
    i16 = mybir.dt.int16
    npad, nblk, e_pad = cfg["npad"], cfg["nblk"], cfg["e_pad"]
    runlen, run_off = cfg["runlen"], cfg["run_off"]
    NBG, BKT = cfg["nbg"], cfg["bkt"]
    NT = e_pad // 2048
    NCHUNK = e_pad // CHUNK

    nc = bacc.Bacc(None, num_devices=n_cores)

    xT_d = nc.dram_tensor("xT", [DIM, npad], f32, kind="ExternalInput")
    xb_d = nc.dram_tensor("xb", [NBG, DIM], bf16, kind="ExternalInput")
    poly_d = nc.dram_tensor("poly", [NT, 6, 1024], f32, kind="ExternalInput")
    seg_d = nc.dram_tensor("segid", [128, e_pad // 128], bf16,
                           kind="ExternalInput")
    sidx_d = nc.dram_tensor("srcidx", [128, e_pad // 16], i16,
                            kind="ExternalInput")
    w_d = {}
    for name, shape, dt in [
            ("lin1_wT", [DIM, NF], bf16), ("lin2_wT", [NF, DIM], bf16),
            ("enn1_wT", [114, NF], bf16), ("enn2_wT", [128, NF], bf16),
            ("poly_lhsT", [6, 128], f32), ("eb", [128, 1], f32),
            ("b1s", [128, 1], f32), ("b2s", [128, 1], f32),
            ("l2b", [128, 1], f32), ("identb", [128, 128], bf16),
            ("ident", [128, 128], f32), ("iota_f", [128, 128], bf16),
            ("iota_f2", [128, 128], bf16)]:
        w_d[name] = nc.dram_tensor(name, shape, dt, kind="ExternalInput")
    out_d = nc.dram_tensor("out", [DIM, npad], f32, kind="ExternalOutput")

    # bucket of each chunk + real (non-pad) slots per chunk (compile-time)
    chunk_bucket = []
    nreal = []
    for bb in range(N_BUCKETS):
        tot = int(runlen[bb].sum())
        sz = int(cfg["bucket_pad"][bb])
        for gg in range(sz // CHUNK):
            chunk_bucket.append(bb)
            nreal.append(min(CHUNK, max(0, tot - gg * CHUNK)))
    assert len(nreal) == NCHUNK

    # compile-time run table: for each chunk, list of
    # (tile, dblock, first, last, parity) spans.
    spans_by_chunk = [[] for _ in range(NCHUNK)]
    last_chunk_of_blk = [0] * nblk
    run_i = 0
    for b in range(N_BUCKETS):
        for k in range(nblk):
            L = int(runlen[b, k])
            if L == 0:
                run_i += 1
                continue
            o = int(run_off[b, k])
            first = True
            pos = o
            while pos < o + L:
                t128 = pos // 128
                k0 = pos % 128
                k1 = min(128, k0 + (o + L - pos))
                g = (t128 * 128) // CHUNK
                pos += k1 - k0
                last = pos >= o + L
                spans_by_chunk[g].append(
                    (t128 - g * 32, k, first, last, run_i % 2))
                first = False
            last_chunk_of_blk[k] = max(last_chunk_of_blk[k],
                                       ((o + L - 1) // 128 * 128) // CHUNK)
            run_i += 1

    # y step s (dst blocks 2s, 2s+1) can run once this chunk is computed
    ysteps_by_chunk = {}
    for s in range(npad // 256):
        blks = [min(2 * s, nblk - 1), min(2 * s + 1, nblk - 1)]
        g = max(last_chunk_of_blk[b_] for b_ in blks)
        ysteps_by_chunk.setdefault(g, []).append(s)

    with tile.TileContext(nc) as tc:
        with (tc.tile_pool(name="const", bufs=1) as cp,
              tc.tile_pool(name="pg", bufs=AHEAD + 1) as pg,
              tc.tile_pool(name="ep", bufs=2) as ep,
              tc.tile_pool(name="yp", bufs=3) as yp,
              tc.tile_pool(name="pp_a", bufs=1, space="PSUM") as pp_a,
              tc.tile_pool(name="pp_b", bufs=1, space="PSUM") as pp_b,
              tc.tile_pool(name="pp_c", bufs=1, space="PSUM") as pp_c,
              tc.tile_pool(name="pp_g", bufs=1, space="PSUM") as pp_g,
              tc.tile_pool(name="pp_s", bufs=2, space="PSUM") as pp_s):
            wt = {}
            for name in w_d:
                t = cp.tile(list(w_d[name].shape), w_d[name].dtype,
                            tag='w_' + name)
                nc.sync.dma_start(out=t[:], in_=w_d[name][:, :])
                wt[name] = t
            nc.gpsimd.load_library(library_config.mlp)
            agg = cp.tile([128, nblk, NF], f32, tag='agg')
            nc.vector.memset(agg[:], 0.0)

            # ---------------- edge phase ----------------
            pend = {}

            def load_chunk(g):
                b = chunk_bucket[g]
                sidx = pg.tile([128, 256], i16, tag='sidx')
                nc.sync.dma_start(
                    out=sidx[:], in_=sidx_d[:, g * 256:(g + 1) * 256])
                segt = pg.tile([128, 32], bf16, tag='segt')
                nc.sync.dma_start(
                    out=segt[:], in_=seg_d[:, g * 32:(g + 1) * 32])
                # transpose-mode gather of x rows: gx[d, slot] = x[src, d]
                gx = pg.tile([128, 1, CHUNK], bf16, tag='gx')
                nidx = ((nreal[g] + 127) // 128) * 128
                if nidx < CHUNK:
                    nc.vector.memset(gx[:, :, nidx:], 0.0)
                if nidx > 0:
                    nc.gpsimd.dma_gather(
                        gx[:, :, :nidx],
                        xb_d[b * BKT:(b + 1) * BKT, :],
                        sidx[:, :nidx // 16],
                        num_idxs=nidx, num_idxs_reg=nidx, elem_size=DIM,
                        transpose=True, single_packet=False)
                pend[g] = (gx, segt)

            agg_ps = {}   # dblock -> psum tile (open accumulations)

            def compute_chunk(g):
                gx, segt = pend.pop(g)
                msg = ep.tile([128, 32, NF], bf16, tag='msg')
                for half in range(2):
                    if half * 2048 >= nreal[g]:
                        continue
                    t = 2 * g + half
                    poly = ep.tile([6, 1024], f32, tag='poly')
                    nc.sync.dma_start(out=poly[:], in_=poly_d[t, :, :])
                    ppsum = pp_a.tile([128, 1024], f32, tag='ppsum')
                    for n5 in range(2):
                        nc.tensor.matmul(
                            ppsum[:, n5 * 512:(n5 + 1) * 512],
                            lhsT=wt["poly_lhsT"][:],
                            rhs=poly[:, n5 * 512:(n5 + 1) * 512],
                            start=True, stop=True)
                    smear = ep.tile([128, 1024], bf16, tag='smear')
                    nc.scalar.activation(
                        smear[:], ppsum[:],
                        mybir.ActivationFunctionType.Exp, bias=wt["eb"][:])
                    h1p = pp_b.tile([128, 1024], f32, tag='h1p')
                    for sub in range(2):
                        for n5 in range(2):
                            nc.tensor.matmul(
                                h1p[sub * 64:(sub + 1) * 64,
                                    n5 * 512:(n5 + 1) * 512],
                                lhsT=wt["enn1_wT"][sub * 64:sub * 64 + NG, :],
                                rhs=smear[sub * 64:sub * 64 + NG,
                                          n5 * 512:(n5 + 1) * 512],
                                start=True, stop=True)
                    h1r = ep.tile([128, 1024], bf16, tag='h1r')
                    nc.scalar.activation(
                        h1r[:], h1p[:], mybir.ActivationFunctionType.Relu,
                        bias=wt["b1s"][:])
                    wtp = pp_a.tile([128, 1024], f32, tag='ppsum', name='wtp')
                    for sub in range(2):
                        for n5 in range(2):
                            nc.tensor.matmul(
                                wtp[sub * 64:(sub + 1) * 64,
                                    n5 * 512:(n5 + 1) * 512],
                                lhsT=wt["enn2_wT"][sub * 64:(sub + 1) * 64, :],
                                rhs=h1r[sub * 64:(sub + 1) * 64,
                                        n5 * 512:(n5 + 1) * 512],
                                start=True, stop=True)
                    wts = ep.tile([128, 1024], bf16, tag='wts')
                    nc.scalar.activation(
                        wts[:], wtp[:],
                        mybir.ActivationFunctionType.Identity,
                        bias=wt["b2s"][:])
                    wcp = pp_c.tile([128, 1024], bf16, tag='wcp')
                    for c in range(8):
                        nc.tensor.transpose(
                            wcp[:, c * 128:(c + 1) * 128],
                            wts[:, c * 128:(c + 1) * 128], wt["identb"][:])
                    wcv = wcp[:].rearrange("p (c k) -> p c k", k=128)
                    for sub in range(2):
                        j0 = half * 16 + sub * 8
                        # h rows for these 8 tiles: h[slot, f] = sum_d
                        # gx[d, slot] * lin1_wT[d, f], per 128-slot tile
                        hg = pp_g.tile([128, 512], f32, tag='hg',
                                       name=f'hg_{g}_{half}_{sub}')
                        for t8 in range(8):
                            t = j0 + t8
                            nc.tensor.matmul(
                                hg[:, t8 * 64:(t8 + 1) * 64],
                                lhsT=gx[:, 0, t * 128:(t + 1) * 128],
                                rhs=wt["lin1_wT"][:],
                                start=True, stop=True)
                        hgs = ep.tile([128, 8, NF], f32, tag='hgs')
                        nc.scalar.activation(
                            hgs[:], hg[:].rearrange("p (c f) -> p c f", f=NF),
                            mybir.ActivationFunctionType.Copy)
                        mslice = msg[:, j0:j0 + 8, :]
                        nc.vector.tensor_tensor(
                            out=mslice,
                            in0=hgs[:],
                            in1=wcv[:, :, sub * 64:(sub + 1) * 64],
                            op=mybir.AluOpType.mult)
                # sel generation (both parities): [128, 32, 128] bf16
                sel0 = ep.tile([128, 32, 128], bf16, tag='sel0')
                nc.vector.tensor_tensor(
                    out=sel0[:],
                    in0=wt["iota_f"][:].unsqueeze(1)
                        .to_broadcast([128, 32, 128]),
                    in1=segt[:].unsqueeze(2)
                        .to_broadcast([128, 32, 128]),
                    op=mybir.AluOpType.is_equal)
                sel1 = ep.tile([128, 32, 128], bf16, tag='sel1')
                nc.vector.tensor_tensor(
                    out=sel1[:],
                    in0=wt["iota_f2"][:].unsqueeze(1)
                        .to_broadcast([128, 32, 128]),
                    in1=segt[:].unsqueeze(2)
                        .to_broadcast([128, 32, 128]),
                    op=mybir.AluOpType.is_equal)
                # sel-reduce with immediate flush at each run's end
                for (t, db, first, last, par) in spans_by_chunk[g]:
                    if first:
                        psnew = pp_s.tile([128, 256], f32, tag='aggps',
                                          name=f'aggps_{g}_{t}_{db}')[:, :NF]
                        agg_ps[db] = psnew
                    ps = agg_ps[db]
                    sel = sel1 if par else sel0
                    nc.tensor.matmul(
                        ps[:],
                        lhsT=sel[:, t, :],
                        rhs=msg[:, t, :],
                        start=first, stop=last)
                    if last:
                        agg_ps.pop(db)
                        nc.vector.tensor_tensor(
                            out=agg[:, db, :], in0=agg[:, db, :],
                            in1=ps[:], op=mybir.AluOpType.add)

            # y PSUM comes exclusively from pp_b (every WAR points backward
            # in program order - never pp_s, whose open aggps accumulators
            # would deadlock the in-order PE queue).
            def y_step(s):
                atp = pp_b.tile([128, 1024], f32, tag='h1p',
                                name=f'atp_{s}')[:64, :256]
                for j in range(2):
                    blk = 2 * s + j
                    nc.tensor.transpose(
                        atp[:, j * 128:(j + 1) * 128],
                        agg[:, blk, :], wt["ident"][:])
                ats = yp.tile([64, 256], bf16, tag='ats')
                nc.scalar.activation(ats[:], atp[:],
                                     mybir.ActivationFunctionType.Copy)
                ytp = pp_b.tile([128, 1024], f32, tag='h1p',
                                name=f'ytp_{s}')[:, :256]
                nc.tensor.matmul(ytp[:], lhsT=wt["lin2_wT"][:],
                                 rhs=ats[:], start=True, stop=True)
                yr = yp.tile([128, 256], f32, tag='yr')
                nc.scalar.activation(yr[:], ytp[:],
                                     mybir.ActivationFunctionType.Relu,
                                     bias=wt["l2b"][:])
                xt2 = yp.tile([128, 256], f32, tag='xt2')
                nc.sync.dma_start(out=xt2[:],
                                  in_=xT_d[:, s * 256:(s + 1) * 256])
                ot = yp.tile([128, 256], f32, tag='ot')
                nc.vector.tensor_tensor(out=ot[:], in0=yr[:], in1=xt2[:],
                                        op=mybir.AluOpType.add)
                nc.sync.dma_start(out=out_d[:, s * 256:(s + 1) * 256],
                                  in_=ot[:])

            # y steps burst only once ALL gathers are issued (iteration
            # >= NCHUNK - AHEAD): injected engine work can no longer stall
            # the gather stream, and the burst hides under the final
            # gathers' execution window.
            ready_s = 0
            for g in range(min(AHEAD, NCHUNK)):
                load_chunk(g)
            for g in range(NCHUNK):
                if g + AHEAD < NCHUNK:
                    load_chunk(g + AHEAD)
                compute_chunk(g)
                if g >= NCHUNK - AHEAD:
                    while (ready_s < npad // 256 and
                           max(last_chunk_of_blk[min(2 * ready_s, nblk - 1)],
                               last_chunk_of_blk[
                                   min(2 * ready_s + 1, nblk - 1)]) <= g):
                        y_step(ready_s)
                        ready_s += 1

            for s in range(ready_s, npad // 256):
                y_step(s)
    nc.compile()
    return nc


def run(inputs, n_cores=N_CORES, trace=False, **_ignored):
    from concourse.bass_utils import run_bass_kernel_spmd

    x = np.asarray(inputs["x"], np.float32)
    cfg, per_core = prep_host(x, inputs["edge_index"], inputs["edge_weight"],
                              inputs["edge_attr"], n_cores)
    wts = prep_weights(cfg, x, inputs["lin1_w"], inputs["lin2_w"],
                       inputs["lin2_b"], inputs["enn1_w"], inputs["enn1_b"],
                       inputs["enn2_w"], inputs["enn2_b"])
    nc = build_nc(cfg, n_cores)
    in_maps = [dict(per_core[r], **wts) for r in range(n_cores)]
    res = run_bass_kernel_spmd(nc, in_maps, core_ids=list(range(n_cores)),
                               trace=trace)
    npc = cfg["npc"]
    out = np.concatenate(
        [np.asarray(res.results[r]["out"]).reshape(DIM, cfg["npad"])[:, :npc].T
         for r in range(n_cores)], axis=0)
    return out, res


def kernel(**inputs):
    out, _ = run(inputs)
    return out
